# revision 39
# baseline (speedup 1.0000x reference)
"""CenterNet (CtdetLoss) Trainium2 Bass kernel.

Math: with p = pred_hm, t = log1p(-p) * p^2, m4 = (1-hm)^4,
  F - Z = t*(m4-1)  densely, plus  ln(p)*(1-p)^2  at the K-sparse
  positive pixels (hm == 1.0, the object centers).

Per-object rectangle sums without SATs. The class selection happens
INSIDE the y-contraction matmul: for each channel group g of CG=8
channels, 8 matmuls with gated weights MyT_cc[y,k] = My[k,y] *
(cls_k % 8 == cc) accumulate into ONE [K,W] PSUM tile, so row k ends
up holding sum_y My[k,y]*g4[y, (cls_k%8)*W + x]. A single
scalar_tensor_tensor with per-partition scalar gate (cls_k//8 == g)
and the Mx mask then reduces it to the group's rect contribution
V10[:, g]; summing over g leaves exactly channel cls_k's rect.
The class-summed Z map (S_ZS) folds all 80 channels onto the same
[K,W] PSUM through ungated myt matmuls. Positive-pixel values come
from an indirect row-gather; the reg-L1 values use one-hot row
matmuls (EY) + one-hot column mask reduces.

Engine split per channel group ([128, 1024] tiles):
  ACT:  l1 = log1p(-p), m2 = (1-hm)^2     (bf16 out)
  Pool: p2 = p*p                          (bf16 out)
  DVE:  t = l1*p2, m4 = m2*m2, g4 = (m4-1)*t   (bf16, 2x rate)
  PE:   8 gated V matmuls + 8 psz matmuls per group
  DVE:  1 gated mask-reduce per group

All per-image constants ship as two packed blobs (one bf16, one f32
with rpos carried as raw int bits) so const loading costs 2 HWDGE
descriptors per image instead of ~19; blob loads and the whole
epilogue input path (pos row-gather, reg-L1 maps) are hoisted ahead
of the dense loop to keep the DMA engines saturated end to end.

Sharding: data-parallel over batch, 2 images per core on 8 cores.
Host combines the per-image partial sums into the final 4 scalars.
"""

import sys

sys.path.insert(0, "/opt/trn_rl_repo")

import numpy as np
import ml_dtypes

B, C, H, W, K = 16, 80, 128, 128, 128
NCORES = 8
NB = B // NCORES          # images per core
CG = 8                    # channels per group
NG = C // CG              # channel groups
HM_W, WH_W, OFF_W = 1.0, 0.1, 1.0

BF16 = ml_dtypes.bfloat16

# blob32 column layout
_MX, _EY, _MT, _CXS, _CSI = 0, 128, 256, 384, 512
_GG, _SK, _M2, _TMW, _TMR, _RPOS = 640, 650, 651, 653, 655, 657
_B32 = 658
# blob16 column layout: mycc cc*128, myt at 1024
_B16 = 1152

_module_cache = {}


def build_module():
    """Build (once) the per-core Bass module."""
    if "nc" in _module_cache:
        return _module_cache["nc"]

    import concourse.bacc as bacc
    import concourse.bass as bass
    import concourse.tile as tile
    from concourse import mybir

    f32 = mybir.dt.float32
    bf16 = mybir.dt.bfloat16
    i32 = mybir.dt.int32
    Alu = mybir.AluOpType
    Act = mybir.ActivationFunctionType
    Ax = mybir.AxisListType

    nc = bacc.Bacc(None, target_bir_lowering=False)

    # ---- DRAM I/O ----
    phm = nc.dram_tensor("phm", [NB, C, H, W], f32, kind="ExternalInput")
    hmt = nc.dram_tensor("hm", [NB, C, H, W], f32, kind="ExternalInput")
    pwh = nc.dram_tensor("pwh", [NB, 2, H, W], f32, kind="ExternalInput")
    prg = nc.dram_tensor("prg", [NB, 2, H, W], f32, kind="ExternalInput")
    blob16 = nc.dram_tensor("blob16", [NB, 128, _B16], bf16, kind="ExternalInput")
    blob32 = nc.dram_tensor("blob32", [NB, 128, _B32], f32, kind="ExternalInput")
    out = nc.dram_tensor("out", [4, NB], f32, kind="ExternalOutput")

    phm_flat = phm[:].rearrange("b c y x -> (b c y) x")

    with tile.TileContext(nc) as tc:
        with (
            tc.tile_pool(name="consts", bufs=1) as consts,
            tc.tile_pool(name="work", bufs=3) as work,
            tc.tile_pool(name="scr", bufs=4) as scr,
            tc.tile_pool(name="acc", bufs=1) as acc,
            tc.tile_pool(name="ep", bufs=2) as ep,
            tc.tile_pool(name="psv", bufs=2, space="PSUM") as psv,
            tc.tile_pool(name="psz", bufs=1, space="PSUM") as pszp,
            tc.tile_pool(name="pss", bufs=1, space="PSUM") as pss,
        ):
            ones_s = consts.tile([K, 1], f32, tag="ones")
            nc.vector.memset(ones_s, 1.0)
            O = acc.tile([4, NB], f32, tag="O")

            # ---- startup: first dense loads lead, blobs interleave ----
            def load_group(b, cs, cgw=CG, hm_first=False):
                def ld_p():
                    p4 = work.tile([H, cgw * W], f32, tag="p4")
                    nc.sync.dma_start(
                        out=p4[:].rearrange("p (c x) -> p c x", c=cgw),
                        in_=phm[b, cs : cs + cgw].rearrange("c y x -> y c x"),
                    )
                    return p4

                def ld_h():
                    hm4 = work.tile([H, cgw * W], f32, tag="hm4")
                    nc.sync.dma_start(
                        out=hm4[:].rearrange("p (c x) -> p c x", c=cgw),
                        in_=hmt[b, cs : cs + cgw].rearrange("c y x -> y c x"),
                    )
                    return hm4

                if hm_first:
                    hm4 = ld_h()
                    p4 = ld_p()
                else:
                    p4 = ld_p()
                    hm4 = ld_h()
                return p4, hm4

            pending = {}
            b16 = []
            b32 = []
            pending[(0, 0)] = load_group(0, 0)
            for b in range(NB):
                t16 = consts.tile([128, _B16], bf16, tag=f"b16_{b}")
                nc.sync.dma_start(out=t16, in_=blob16[b])
                t32 = consts.tile([128, _B32], f32, tag=f"b32_{b}")
                nc.sync.dma_start(out=t32, in_=blob32[b])
                b16.append(t16)
                b32.append(t32)
                if b == 0:
                    pending[(0, 1)] = load_group(0, CG)
            # positive-pixel row gathers (SWDGE; independent of dense loop)
            rowg_s = []
            for b in range(NB):
                rowg = consts.tile([K, W], f32, tag=f"rowg{b}")
                nc.gpsimd.indirect_dma_start(
                    out=rowg,
                    out_offset=None,
                    in_=phm_flat,
                    in_offset=bass.IndirectOffsetOnAxis(
                        ap=b32[b][:, _RPOS : _RPOS + 1].bitcast(i32), axis=0
                    ),
                )
                rowg_s.append(rowg)

            # reg-L1 map loads: early in the DMA stream (behind the two
            # prefetched dense groups) so the data waits in SBUF; the
            # dependent math runs mid-stream via epi_math
            pw_s = {}
            for b in range(NB):
                for col, dmap in ((1, pwh), (2, prg)):
                    pw = consts.tile([H, 2 * W], f32, tag=f"pw{b}{col}")
                    nc.sync.dma_start(
                        out=pw[:].rearrange("p (d x) -> p d x", d=2),
                        in_=dmap[b].rearrange("d y x -> y d x"),
                    )
                    pw_s[(b, col)] = pw

            Qs = []
            for b in range(NB):
                Q = ep.tile([K, 4], f32, tag=f"Q{b}")
                nc.vector.memset(Q, 0.0)
                Qs.append(Q)
            psp_s = {}

            def epi_math(b):
                # pos-term + reg-L1 math, emitted mid-stream (after the
                # first dense loop) — all inputs arrived long ago, so
                # these never block the in-order ACT/DVE queues.
                t32 = b32[b]
                Q = Qs[b]
                pj = ep.tile([K, 1], f32, tag=f"pj{b}")
                scp = scr.tile([K, W], f32, tag="scrP")
                nc.vector.scalar_tensor_tensor(
                    scp, rowg_s[b], 1.0, t32[:, _CXS : _CXS + W],
                    op0=Alu.mult, op1=Alu.mult, accum_out=pj,
                )
                lnp = ep.tile([K, 1], f32, tag=f"lnp{b}")
                nc.scalar.activation(lnp, pj, Act.Ln)
                q2 = ep.tile([K, 1], f32, tag=f"q2{b}")
                nc.scalar.activation(q2, pj, Act.Square, bias=1.0, scale=-1.0)
                A = ep.tile([K, 1], f32, tag=f"A{b}")
                nc.vector.tensor_mul(A, lnp, q2)
                psp = pss.tile([K, 1], f32, tag="psp")
                nc.tensor.matmul(
                    psp, lhsT=t32[:, _MT : _MT + K], rhs=A, start=True, stop=True
                )
                psp_s[b] = psp
                for col in (1, 2):
                    Tw = pss.tile([K, 2 * W], f32, tag="Tw")
                    nc.tensor.matmul(
                        Tw, lhsT=t32[:, _EY : _EY + K], rhs=pw_s[(b, col)],
                        start=True, stop=True,
                    )
                    PW = ep.tile([K, 2], f32, tag=f"PW{b}{col}")
                    for d in range(2):
                        scw = scr.tile([K, W], f32, tag="scrW")
                        nc.vector.scalar_tensor_tensor(
                            scw, Tw[:, d * W : d * W + W], 1.0,
                            t32[:, _CSI : _CSI + W],
                            op0=Alu.mult, op1=Alu.mult,
                            accum_out=PW[:, d : d + 1],
                        )
                    tmoff = _TMW if col == 1 else _TMR
                    u = ep.tile([K, 2], f32, tag=f"u{b}{col}")
                    nc.vector.tensor_mul(u, PW, t32[:, _M2 : _M2 + 2])
                    nc.vector.tensor_sub(u, u, t32[:, tmoff : tmoff + 2])
                    nc.vector.tensor_reduce(
                        Q[:, col : col + 1], u, axis=Ax.X, op=Alu.add,
                        apply_absolute_value=True,
                    )

            # ---- dense loops ----
            for b in range(NB):
                t16 = b16[b]
                t32 = b32[b]
                # image 1's last group tapers to [6,1,1] channels so the
                # post-DMA drain chain operates on tiny tiles
                if b == 0:
                    chunks = [(g, g * CG, CG) for g in range(NG)]
                else:
                    chunks = [(g, g * CG, CG) for g in range(NG - 1)]
                    cs9 = (NG - 1) * CG
                    chunks += [(NG - 1, cs9, CG // 2),
                               (NG - 1, cs9 + CG // 2, CG // 2)]
                V10 = acc.tile([K, len(chunks)], f32, tag=f"V10{b}")
                psz_acc = pszp.tile([K, W], f32, tag=f"pszacc{b}")

                prev_stt = None
                for ci, (g, cs, cgw) in enumerate(chunks):
                    tailc = cgw != CG  # tapered tail chunk: m-chain first
                    if (b, ci) in pending:
                        p4, hm4 = pending.pop((b, ci))
                    else:
                        p4, hm4 = load_group(b, cs, cgw, hm_first=tailc)
                    # ACT: l1 = log1p(-p), m2 = (1-hm)^2
                    l1 = work.tile([H, cgw * W], bf16, tag="l1")
                    m2t = work.tile([H, cgw * W], bf16, tag="m2t")
                    if tailc:
                        nc.scalar.activation(
                            m2t, hm4, Act.Square, bias=1.0, scale=-1.0
                        )
                        nc.scalar.activation(l1, p4, Act.Ln, bias=1.0, scale=-1.0)
                    else:
                        nc.scalar.activation(l1, p4, Act.Ln, bias=1.0, scale=-1.0)
                        nc.scalar.activation(
                            m2t, hm4, Act.Square, bias=1.0, scale=-1.0
                        )
                    # Pool: p2 = p*p
                    p2 = work.tile([H, cgw * W], bf16, tag="p2")
                    nc.gpsimd.tensor_mul(p2, p4, p4)
                    # DVE: t = l1*p2 ; m4 = m2^2 ; g4 = (m4-1)*t
                    m4t = work.tile([H, cgw * W], bf16, tag="m4t")
                    m4m1 = work.tile([H, cgw * W], bf16, tag="m4m1")
                    t = work.tile([H, cgw * W], bf16, tag="t")
                    if tailc:
                        nc.vector.tensor_mul(m4t, m2t, m2t)
                        nc.vector.tensor_scalar(m4m1, m4t, -1.0, None, op0=Alu.add)
                        nc.vector.tensor_mul(t, l1, p2)
                    else:
                        nc.vector.tensor_mul(t, l1, p2)
                        nc.vector.tensor_mul(m4t, m2t, m2t)
                        nc.vector.tensor_scalar(m4m1, m4t, -1.0, None, op0=Alu.add)
                    g4 = work.tile([H, cgw * W], bf16, tag="g4")
                    nc.vector.tensor_mul(g4, m4m1, t)
                    # deferred V-stt of the PREVIOUS chunk: by now its PE
                    # matmuls are long done, so DVE never stalls on PE
                    if prev_stt is not None:
                        pg, pgate, pci = prev_stt
                        sc = scr.tile([K, W], f32, tag="scrV")
                        nc.vector.scalar_tensor_tensor(
                            sc, pg, t32[:, _GG + pgate : _GG + pgate + 1],
                            t32[:, _MX : _MX + W],
                            op0=Alu.mult, op1=Alu.mult,
                            accum_out=V10[:, pci : pci + 1],
                        )
                    # PE: psz_acc += myt.T @ t   (all channels fold onto W)
                    for cc in range(cgw):
                        nc.tensor.matmul(
                            psz_acc, lhsT=t16[:, 1024 : 1024 + K],
                            rhs=t[:, cc * W : cc * W + W],
                            start=(ci == 0 and cc == 0),
                            stop=(ci == len(chunks) - 1 and cc == cgw - 1),
                            skip_group_check=True,
                        )
                    # PE: gated V matmuls — row k accumulates its own
                    # channel's (cls_k % CG) y-contraction of g4
                    psg = psv.tile([K, W], f32, tag="psg")
                    for cc in range(cgw):
                        mcc = (cs + cc) % CG
                        nc.tensor.matmul(
                            psg, lhsT=t16[:, mcc * K : mcc * K + K],
                            rhs=g4[:, cc * W : cc * W + W],
                            start=(cc == 0), stop=(cc == cgw - 1),
                        )
                    prev_stt = (psg, g, ci)

                # flush the final chunk's deferred V-stt
                pg, pgate, pci = prev_stt
                sc = scr.tile([K, W], f32, tag="scrV")
                nc.vector.scalar_tensor_tensor(
                    sc, pg, t32[:, _GG + pgate : _GG + pgate + 1],
                    t32[:, _MX : _MX + W],
                    op0=Alu.mult, op1=Alu.mult,
                    accum_out=V10[:, pci : pci + 1],
                )

                if b == 0:
                    for bb in range(NB):
                        epi_math(bb)

                # ---- per-image combine ----
                szs = ep.tile([K, 1], f32, tag=f"szs{b}")
                scz = scr.tile([K, W], f32, tag="scrZ")
                nc.vector.scalar_tensor_tensor(
                    scz, psz_acc, 1.0, t32[:, _MX : _MX + W],
                    op0=Alu.mult, op1=Alu.mult, accum_out=szs,
                )
                rectG = ep.tile([K, 1], f32, tag=f"rectG{b}")
                nc.vector.tensor_reduce(rectG, V10, axis=Ax.X, op=Alu.add)
                tot = ep.tile([K, 1], f32, tag=f"tot{b}")
                nc.vector.tensor_add(tot, rectG, psp_s[b])
                nc.vector.tensor_add(tot, tot, szs)
                Q = Qs[b]
                nc.vector.tensor_mul(Q[:, 0:1], tot, t32[:, _SK : _SK + 1])
                psq = pss.tile([4, 1], f32, tag="psq")
                nc.tensor.matmul(psq, lhsT=Q, rhs=ones_s, start=True, stop=True)
                nc.scalar.copy(O[:, b : b + 1], psq)

            nc.sync.dma_start(out=out[:], in_=O)

    nc.compile()
    _module_cache["nc"] = nc
    return nc


def prep_in_maps(inputs):
    """Host-side prep: shard the dense maps per core, derive mask/index
    constants from the small int tensors, pack them into blobs."""
    pred_hm = np.asarray(inputs["pred_hm"], np.float32)
    pred_wh = np.asarray(inputs["pred_wh"], np.float32)
    pred_reg = np.asarray(inputs["pred_reg"], np.float32)
    hm = np.asarray(inputs["hm"], np.float32)
    wh_t = np.asarray(inputs["wh_t"], np.float32)
    reg_t = np.asarray(inputs["reg_t"], np.float32)
    reg_mask = np.asarray(inputs["reg_mask"], np.float32)
    ind = np.asarray(inputs["ind"]).astype(np.int64)
    cxcy = np.asarray(inputs["cxcy"]).astype(np.int64)
    ori_wh = np.asarray(inputs["ori_wh"]).astype(np.int64)
    cls_idx = np.asarray(inputs["cls_idx"]).astype(np.int64)

    yy = np.arange(H)
    xx = np.arange(W)
    per_img = []
    for b in range(B):
        cls = cls_idx[b]
        cx, cy = cxcy[b, :, 0], cxcy[b, :, 1]
        w = wh_t[b, :, 0].astype(np.int64)
        h = wh_t[b, :, 1].astype(np.int64)
        y0 = np.maximum(1, cy - h // 2 - 1)
        y1 = np.minimum(H - 1, cy + h // 2 + 1)
        y1 = np.maximum(y1, y0)
        x0 = np.maximum(1, cx - w // 2 - 1)
        x1 = np.minimum(W - 1, cx + w // 2 + 1)
        x1 = np.maximum(x1, x0)

        MyT = ((yy[:, None] >= y0[None, :]) & (yy[:, None] < y1[None, :]))
        Mx = ((xx[None, :] >= x0[:, None]) & (xx[None, :] < x1[:, None]))

        blob16 = np.zeros((128, _B16), np.float32)
        for cc in range(CG):
            blob16[:, cc * K : cc * K + K] = MyT * (cls[None, :] % CG == cc)
        blob16[:, 1024 : 1024 + K] = MyT

        aspect = w.astype(np.float32) / h.astype(np.float32)
        ori = ori_wh[b, :, 0].astype(np.float32) / ori_wh[b, :, 1].astype(np.float32)
        bad = ~((aspect > 0.5 * ori) & (aspect < 2.0 * ori))
        badw = np.where(bad, 0.5, 1.0).astype(np.float32)
        valid = reg_mask[b] * (w * h > 0).astype(np.float32)

        # unique positive pixels (duplicated centers collapse in hm)
        flat = cls * (H * W) + cy * W + cx
        _, uidx = np.unique(flat, return_index=True)
        nu = len(uidx)
        cls_u, cy_u, cx_u = cls[uidx], cy[uidx], cx[uidx]
        inY = (cy_u[None, :] >= y0[:, None]) & (cy_u[None, :] < y1[:, None])
        inX = (cx_u[None, :] >= x0[:, None]) & (cx_u[None, :] < x1[:, None])
        sameC = cls[:, None] == cls_u[None, :]
        Mkj = (sameC & inY & inX).astype(np.float32)  # [k, j<nu]
        npos = Mkj.sum(1)
        MT = np.zeros((K, K), np.float32)
        MT[:nu, :] = Mkj.T
        rpos_v = np.zeros(K, np.int32)
        rpos_v[:nu] = (b % NB) * C * H + cls_u * H + cy_u
        cxsel_v = np.zeros((K, W), np.float32)
        cx_pad = np.zeros(K, np.int64)
        cx_pad[:nu] = cx_u
        cxsel_v[np.arange(K), cx_pad] = 1.0

        r = np.where(npos > 0, 1.0 / np.maximum(npos, 1.0), 1.0)
        s = (-(r * badw * valid)).astype(np.float32)

        rind = ind[b] // W
        cind = ind[b] % W
        EY = (yy[:, None] == rind[None, :]).astype(np.float32)  # [H, K]
        csind_v = np.zeros((K, W), np.float32)
        csind_v[np.arange(K), cind] = 1.0

        m = reg_mask[b]
        blob32 = np.zeros((128, _B32), np.float32)
        blob32[:, _MX : _MX + W] = Mx
        blob32[:, _EY : _EY + K] = EY
        blob32[:, _MT : _MT + K] = MT
        blob32[:, _CXS : _CXS + W] = cxsel_v
        blob32[:, _CSI : _CSI + W] = csind_v
        blob32[:, _GG : _GG + NG] = (
            cls[:, None] // CG == np.arange(NG)[None, :]
        ).astype(np.float32)
        blob32[:, _SK] = s
        blob32[:, _M2] = m
        blob32[:, _M2 + 1] = m
        blob32[:, _TMW : _TMW + 2] = wh_t[b] * m[:, None]
        blob32[:, _TMR : _TMR + 2] = reg_t[b] * m[:, None]
        blob32[:, _RPOS] = rpos_v.view(np.float32)

        nobj = float(m.sum())
        c1 = (1.0 / max(nobj, 1.0)) if nobj > 0 else 1.0
        invden = 1.0 / (2.0 * nobj + 1e-4)

        per_img.append(
            dict(
                blob16=blob16.astype(BF16), blob32=blob32, c1=c1, invden=invden
            )
        )

    in_maps = []
    for core in range(NCORES):
        bs = [core * NB + j for j in range(NB)]
        pi = [per_img[b] for b in bs]
        in_maps.append(
            {
                "phm": np.ascontiguousarray(pred_hm[bs]),
                "hm": np.ascontiguousarray(hm[bs]),
                "pwh": np.ascontiguousarray(pred_wh[bs]),
                "prg": np.ascontiguousarray(pred_reg[bs]),
                "blob16": np.stack([p["blob16"] for p in pi]),
                "blob32": np.stack([p["blob32"] for p in pi]),
            }
        )
    aux = dict(
        c1=np.array([p["c1"] for p in per_img]),
        invden=np.array([p["invden"] for p in per_img]),
    )
    return in_maps, aux


def combine_outputs(outs, aux):
    """outs: list of 8 per-core 'out' arrays [4, NB]."""
    q = np.concatenate([o.T for o in outs], 0).astype(np.float64)  # [B, 4]
    q_hm, q_wh, q_rg = q[:, 0], q[:, 1], q[:, 2]
    wh_i = q_wh * aux["invden"]
    off_i = q_rg * aux["invden"]
    final_loss = np.mean(HM_W * q_hm + WH_W * wh_i + OFF_W * off_i)
    final_hm = np.mean(q_hm * aux["c1"])
    final_wh = np.mean(wh_i)
    final_off = np.mean(off_i)
    return (
        np.float32(final_loss),
        np.float32(final_hm),
        np.float32(final_wh),
        np.float32(final_off),
    )


def kernel(**inputs):
    from concourse.bass_utils import run_bass_kernel_spmd

    nc = build_module()
    in_maps, aux = prep_in_maps(inputs)
    res = run_bass_kernel_spmd(nc, in_maps, core_ids=list(range(NCORES)))
    outs = [r["out"] for r in res.results]
    return combine_outputs(outs, aux)


# revision 40
# speedup vs baseline: 1.0266x; 1.0266x over previous
"""CenterNet (CtdetLoss) Trainium2 Bass kernel.

Math: with p = pred_hm, t = log1p(-p) * p^2, m4 = (1-hm)^4,
  F - Z = t*(m4-1)  densely, plus  ln(p)*(1-p)^2  at the K-sparse
  positive pixels (hm == 1.0, the object centers).

Per-object rectangle sums without SATs. The class selection happens
INSIDE the y-contraction matmul: for each channel group g of CG=8
channels, 8 matmuls with gated weights MyT_cc[y,k] = My[k,y] *
(cls_k % 8 == cc) accumulate into ONE [K,W] PSUM tile, so row k ends
up holding sum_y My[k,y]*g4[y, (cls_k%8)*W + x]. A single
scalar_tensor_tensor with per-partition scalar gate (cls_k//8 == g)
and the Mx mask then reduces it to the group's rect contribution
V10[:, g]; summing over g leaves exactly channel cls_k's rect.
The class-summed Z map (S_ZS) folds all 80 channels onto the same
[K,W] PSUM through ungated myt matmuls. Positive-pixel values come
from an indirect row-gather; the reg-L1 values use one-hot row
matmuls (EY) + one-hot column mask reduces.

Engine split per channel group ([128, 1024] tiles):
  ACT:  l1 = log1p(-p), m2 = (1-hm)^2     (bf16 out)
  Pool: p2 = p*p                          (bf16 out)
  DVE:  t = l1*p2, m4 = m2*m2, g4 = (m4-1)*t   (bf16, 2x rate)
  PE:   8 gated V matmuls + 8 psz matmuls per group
  DVE:  1 gated mask-reduce per group

All per-image constants ship as two packed blobs (one bf16, one f32
with rpos carried as raw int bits) so const loading costs 2 HWDGE
descriptors per image instead of ~19; blob loads and the whole
epilogue input path (pos row-gather, reg-L1 maps) are hoisted ahead
of the dense loop to keep the DMA engines saturated end to end.

Sharding: data-parallel over batch, 2 images per core on 8 cores.
Host combines the per-image partial sums into the final 4 scalars.
"""

import sys

sys.path.insert(0, "/opt/trn_rl_repo")

import numpy as np
import ml_dtypes

B, C, H, W, K = 16, 80, 128, 128, 128
NCORES = 8
NB = B // NCORES          # images per core
CG = 8                    # channels per group
NG = C // CG              # channel groups
HM_W, WH_W, OFF_W = 1.0, 0.1, 1.0

BF16 = ml_dtypes.bfloat16

# blob32 column layout
_EY = 0
_GG, _SK, _M2, _TMW, _TMR, _RPOS = 128, 138, 139, 141, 143, 145
_B32 = 146
# blob16 column layout: mycc cc*128, myt at 1024, then masks
_MYT, _MX, _MT, _CXS, _CSI = 1024, 1152, 1280, 1408, 1536
_B16 = 1664

_module_cache = {}


def build_module():
    """Build (once) the per-core Bass module."""
    if "nc" in _module_cache:
        return _module_cache["nc"]

    import concourse.bacc as bacc
    import concourse.bass as bass
    import concourse.tile as tile
    from concourse import mybir

    f32 = mybir.dt.float32
    bf16 = mybir.dt.bfloat16
    i32 = mybir.dt.int32
    Alu = mybir.AluOpType
    Act = mybir.ActivationFunctionType
    Ax = mybir.AxisListType

    nc = bacc.Bacc(None, target_bir_lowering=False)

    # ---- DRAM I/O ----
    phm = nc.dram_tensor("phm", [NB, C, H, W], f32, kind="ExternalInput")
    hmt = nc.dram_tensor("hm", [NB, C, H, W], f32, kind="ExternalInput")
    pwh = nc.dram_tensor("pwh", [NB, 2, H, W], f32, kind="ExternalInput")
    prg = nc.dram_tensor("prg", [NB, 2, H, W], f32, kind="ExternalInput")
    blob16 = nc.dram_tensor("blob16", [NB, 128, _B16], bf16, kind="ExternalInput")
    blob32 = nc.dram_tensor("blob32", [NB, 128, _B32], f32, kind="ExternalInput")
    out = nc.dram_tensor("out", [4, NB], f32, kind="ExternalOutput")

    phm_flat = phm[:].rearrange("b c y x -> (b c y) x")

    with tile.TileContext(nc) as tc:
        with (
            tc.tile_pool(name="consts", bufs=1) as consts,
            tc.tile_pool(name="work", bufs=3) as work,
            tc.tile_pool(name="scr", bufs=4) as scr,
            tc.tile_pool(name="acc", bufs=1) as acc,
            tc.tile_pool(name="ep", bufs=2) as ep,
            tc.tile_pool(name="psv", bufs=2, space="PSUM") as psv,
            tc.tile_pool(name="psz", bufs=1, space="PSUM") as pszp,
            tc.tile_pool(name="pss", bufs=1, space="PSUM") as pss,
        ):
            ones_s = consts.tile([K, 1], f32, tag="ones")
            nc.vector.memset(ones_s, 1.0)
            O = acc.tile([4, NB], f32, tag="O")

            # ---- startup: first dense loads lead, blobs interleave ----
            def load_group(b, cs, cgw=CG, hm_first=False):
                def ld_p():
                    p4 = work.tile([H, cgw * W], f32, tag="p4")
                    nc.sync.dma_start(
                        out=p4[:].rearrange("p (c x) -> p c x", c=cgw),
                        in_=phm[b, cs : cs + cgw].rearrange("c y x -> y c x"),
                    )
                    return p4

                def ld_h():
                    hm4 = work.tile([H, cgw * W], f32, tag="hm4")
                    nc.sync.dma_start(
                        out=hm4[:].rearrange("p (c x) -> p c x", c=cgw),
                        in_=hmt[b, cs : cs + cgw].rearrange("c y x -> y c x"),
                    )
                    return hm4

                if hm_first:
                    hm4 = ld_h()
                    p4 = ld_p()
                else:
                    p4 = ld_p()
                    hm4 = ld_h()
                return p4, hm4

            pending = {}
            b16 = []
            b32 = []
            pending[(0, 0)] = load_group(0, 0)
            for b in range(NB):
                t16 = consts.tile([128, _B16], bf16, tag=f"b16_{b}")
                nc.sync.dma_start(out=t16, in_=blob16[b])
                t32 = consts.tile([128, _B32], f32, tag=f"b32_{b}")
                nc.sync.dma_start(out=t32, in_=blob32[b])
                b16.append(t16)
                b32.append(t32)
                if b == 0:
                    pending[(0, 1)] = load_group(0, CG)
            # positive-pixel row gathers (SWDGE; independent of dense loop)
            rowg_s = []
            for b in range(NB):
                rowg = consts.tile([K, W], f32, tag=f"rowg{b}")
                nc.gpsimd.indirect_dma_start(
                    out=rowg,
                    out_offset=None,
                    in_=phm_flat,
                    in_offset=bass.IndirectOffsetOnAxis(
                        ap=b32[b][:, _RPOS : _RPOS + 1].bitcast(i32), axis=0
                    ),
                )
                rowg_s.append(rowg)

            # reg-L1 map loads: early in the DMA stream (behind the two
            # prefetched dense groups) so the data waits in SBUF; the
            # dependent math runs mid-stream via epi_math
            pw_s = {}
            for b in range(NB):
                for col, dmap in ((1, pwh), (2, prg)):
                    pw = consts.tile([H, 2 * W], f32, tag=f"pw{b}{col}")
                    nc.sync.dma_start(
                        out=pw[:].rearrange("p (d x) -> p d x", d=2),
                        in_=dmap[b].rearrange("d y x -> y d x"),
                    )
                    pw_s[(b, col)] = pw

            Qs = []
            for b in range(NB):
                Q = ep.tile([K, 4], f32, tag=f"Q{b}")
                nc.vector.memset(Q, 0.0)
                Qs.append(Q)
            psp_s = {}

            def epi_math(b):
                # pos-term + reg-L1 math, emitted mid-stream (after the
                # first dense loop) — all inputs arrived long ago, so
                # these never block the in-order ACT/DVE queues.
                t32 = b32[b]
                Q = Qs[b]
                pj = ep.tile([K, 1], f32, tag=f"pj{b}")
                scp = scr.tile([K, W], f32, tag="scrP")
                nc.vector.scalar_tensor_tensor(
                    scp, rowg_s[b], 1.0, t16[:, _CXS : _CXS + W],
                    op0=Alu.mult, op1=Alu.mult, accum_out=pj,
                )
                lnp = ep.tile([K, 1], f32, tag=f"lnp{b}")
                nc.scalar.activation(lnp, pj, Act.Ln)
                q2 = ep.tile([K, 1], f32, tag=f"q2{b}")
                nc.scalar.activation(q2, pj, Act.Square, bias=1.0, scale=-1.0)
                A = ep.tile([K, 1], bf16, tag=f"A{b}")
                nc.vector.tensor_mul(A, lnp, q2)
                psp = pss.tile([K, 1], f32, tag="psp")
                nc.tensor.matmul(
                    psp, lhsT=t16[:, _MT : _MT + K], rhs=A, start=True, stop=True
                )
                psp_s[b] = psp
                for col in (1, 2):
                    Tw = pss.tile([K, 2 * W], f32, tag="Tw")
                    nc.tensor.matmul(
                        Tw, lhsT=t32[:, _EY : _EY + K], rhs=pw_s[(b, col)],
                        start=True, stop=True,
                    )
                    PW = ep.tile([K, 2], f32, tag=f"PW{b}{col}")
                    for d in range(2):
                        scw = scr.tile([K, W], f32, tag="scrW")
                        nc.vector.scalar_tensor_tensor(
                            scw, Tw[:, d * W : d * W + W], 1.0,
                            t16[:, _CSI : _CSI + W],
                            op0=Alu.mult, op1=Alu.mult,
                            accum_out=PW[:, d : d + 1],
                        )
                    tmoff = _TMW if col == 1 else _TMR
                    u = ep.tile([K, 2], f32, tag=f"u{b}{col}")
                    nc.vector.tensor_mul(u, PW, t32[:, _M2 : _M2 + 2])
                    nc.vector.tensor_sub(u, u, t32[:, tmoff : tmoff + 2])
                    nc.vector.tensor_reduce(
                        Q[:, col : col + 1], u, axis=Ax.X, op=Alu.add,
                        apply_absolute_value=True,
                    )

            # ---- dense loops ----
            for b in range(NB):
                t16 = b16[b]
                t32 = b32[b]
                # image 1's last group tapers to [6,1,1] channels so the
                # post-DMA drain chain operates on tiny tiles
                if b == 0:
                    chunks = [(g, g * CG, CG) for g in range(NG)]
                else:
                    chunks = [(g, g * CG, CG) for g in range(NG - 1)]
                    cs9 = (NG - 1) * CG
                    chunks += [(NG - 1, cs9, CG // 2),
                               (NG - 1, cs9 + CG // 2, CG // 2)]
                V10 = acc.tile([K, len(chunks)], f32, tag=f"V10{b}")
                psz_acc = pszp.tile([K, W], f32, tag=f"pszacc{b}")

                prev_stt = None
                for ci, (g, cs, cgw) in enumerate(chunks):
                    tailc = cgw != CG  # tapered tail chunk: m-chain first
                    if (b, ci) in pending:
                        p4, hm4 = pending.pop((b, ci))
                    else:
                        p4, hm4 = load_group(b, cs, cgw, hm_first=tailc)
                    # ACT: l1 = log1p(-p), m2 = (1-hm)^2
                    l1 = work.tile([H, cgw * W], bf16, tag="l1")
                    m2t = work.tile([H, cgw * W], bf16, tag="m2t")
                    if tailc:
                        nc.scalar.activation(
                            m2t, hm4, Act.Square, bias=1.0, scale=-1.0
                        )
                        nc.scalar.activation(l1, p4, Act.Ln, bias=1.0, scale=-1.0)
                    else:
                        nc.scalar.activation(l1, p4, Act.Ln, bias=1.0, scale=-1.0)
                        nc.scalar.activation(
                            m2t, hm4, Act.Square, bias=1.0, scale=-1.0
                        )
                    # Pool: p2 = p*p
                    p2 = work.tile([H, cgw * W], bf16, tag="p2")
                    nc.gpsimd.tensor_mul(p2, p4, p4)
                    # DVE: t = l1*p2 ; m4 = m2^2 ; g4 = (m4-1)*t
                    m4t = work.tile([H, cgw * W], bf16, tag="m4t")
                    m4m1 = work.tile([H, cgw * W], bf16, tag="m4m1")
                    t = work.tile([H, cgw * W], bf16, tag="t")
                    if tailc:
                        nc.vector.tensor_mul(m4t, m2t, m2t)
                        nc.vector.tensor_scalar(m4m1, m4t, -1.0, None, op0=Alu.add)
                        nc.vector.tensor_mul(t, l1, p2)
                    else:
                        nc.vector.tensor_mul(t, l1, p2)
                        nc.vector.tensor_mul(m4t, m2t, m2t)
                        nc.vector.tensor_scalar(m4m1, m4t, -1.0, None, op0=Alu.add)
                    g4 = work.tile([H, cgw * W], bf16, tag="g4")
                    nc.vector.tensor_mul(g4, m4m1, t)
                    # deferred V-stt of the PREVIOUS chunk: by now its PE
                    # matmuls are long done, so DVE never stalls on PE
                    if prev_stt is not None:
                        pg, pgate, pci = prev_stt
                        sc = scr.tile([K, W], f32, tag="scrV")
                        nc.vector.scalar_tensor_tensor(
                            sc, pg, t32[:, _GG + pgate : _GG + pgate + 1],
                            t16[:, _MX : _MX + W],
                            op0=Alu.mult, op1=Alu.mult,
                            accum_out=V10[:, pci : pci + 1],
                        )
                    # PE: psz_acc += myt.T @ t   (all channels fold onto W)
                    for cc in range(cgw):
                        nc.tensor.matmul(
                            psz_acc, lhsT=t16[:, _MYT : _MYT + K],
                            rhs=t[:, cc * W : cc * W + W],
                            start=(ci == 0 and cc == 0),
                            stop=(ci == len(chunks) - 1 and cc == cgw - 1),
                            skip_group_check=True,
                        )
                    # PE: gated V matmuls — row k accumulates its own
                    # channel's (cls_k % CG) y-contraction of g4
                    psg = psv.tile([K, W], f32, tag="psg")
                    for cc in range(cgw):
                        mcc = (cs + cc) % CG
                        nc.tensor.matmul(
                            psg, lhsT=t16[:, mcc * K : mcc * K + K],
                            rhs=g4[:, cc * W : cc * W + W],
                            start=(cc == 0), stop=(cc == cgw - 1),
                        )
                    prev_stt = (psg, g, ci)

                # flush the final chunk's deferred V-stt
                pg, pgate, pci = prev_stt
                sc = scr.tile([K, W], f32, tag="scrV")
                nc.vector.scalar_tensor_tensor(
                    sc, pg, t32[:, _GG + pgate : _GG + pgate + 1],
                    t16[:, _MX : _MX + W],
                    op0=Alu.mult, op1=Alu.mult,
                    accum_out=V10[:, pci : pci + 1],
                )

                if b == 0:
                    for bb in range(NB):
                        epi_math(bb)

                # ---- per-image combine ----
                szs = ep.tile([K, 1], f32, tag=f"szs{b}")
                scz = scr.tile([K, W], f32, tag="scrZ")
                nc.vector.scalar_tensor_tensor(
                    scz, psz_acc, 1.0, t16[:, _MX : _MX + W],
                    op0=Alu.mult, op1=Alu.mult, accum_out=szs,
                )
                rectG = ep.tile([K, 1], f32, tag=f"rectG{b}")
                nc.vector.tensor_reduce(rectG, V10, axis=Ax.X, op=Alu.add)
                tot = ep.tile([K, 1], f32, tag=f"tot{b}")
                nc.vector.tensor_add(tot, rectG, psp_s[b])
                nc.vector.tensor_add(tot, tot, szs)
                Q = Qs[b]
                nc.vector.tensor_mul(Q[:, 0:1], tot, t32[:, _SK : _SK + 1])
                psq = pss.tile([4, 1], f32, tag="psq")
                nc.tensor.matmul(psq, lhsT=Q, rhs=ones_s, start=True, stop=True)
                nc.scalar.copy(O[:, b : b + 1], psq)

            nc.sync.dma_start(out=out[:], in_=O)

    nc.compile()
    _module_cache["nc"] = nc
    return nc


def prep_in_maps(inputs):
    """Host-side prep: shard the dense maps per core, derive mask/index
    constants from the small int tensors, pack them into blobs."""
    pred_hm = np.asarray(inputs["pred_hm"], np.float32)
    pred_wh = np.asarray(inputs["pred_wh"], np.float32)
    pred_reg = np.asarray(inputs["pred_reg"], np.float32)
    hm = np.asarray(inputs["hm"], np.float32)
    wh_t = np.asarray(inputs["wh_t"], np.float32)
    reg_t = np.asarray(inputs["reg_t"], np.float32)
    reg_mask = np.asarray(inputs["reg_mask"], np.float32)
    ind = np.asarray(inputs["ind"]).astype(np.int64)
    cxcy = np.asarray(inputs["cxcy"]).astype(np.int64)
    ori_wh = np.asarray(inputs["ori_wh"]).astype(np.int64)
    cls_idx = np.asarray(inputs["cls_idx"]).astype(np.int64)

    yy = np.arange(H)
    xx = np.arange(W)
    per_img = []
    for b in range(B):
        cls = cls_idx[b]
        cx, cy = cxcy[b, :, 0], cxcy[b, :, 1]
        w = wh_t[b, :, 0].astype(np.int64)
        h = wh_t[b, :, 1].astype(np.int64)
        y0 = np.maximum(1, cy - h // 2 - 1)
        y1 = np.minimum(H - 1, cy + h // 2 + 1)
        y1 = np.maximum(y1, y0)
        x0 = np.maximum(1, cx - w // 2 - 1)
        x1 = np.minimum(W - 1, cx + w // 2 + 1)
        x1 = np.maximum(x1, x0)

        MyT = ((yy[:, None] >= y0[None, :]) & (yy[:, None] < y1[None, :]))
        Mx = ((xx[None, :] >= x0[:, None]) & (xx[None, :] < x1[:, None]))

        blob16 = np.zeros((128, _B16), np.float32)
        for cc in range(CG):
            blob16[:, cc * K : cc * K + K] = MyT * (cls[None, :] % CG == cc)
        blob16[:, _MYT : _MYT + K] = MyT

        aspect = w.astype(np.float32) / h.astype(np.float32)
        ori = ori_wh[b, :, 0].astype(np.float32) / ori_wh[b, :, 1].astype(np.float32)
        bad = ~((aspect > 0.5 * ori) & (aspect < 2.0 * ori))
        badw = np.where(bad, 0.5, 1.0).astype(np.float32)
        valid = reg_mask[b] * (w * h > 0).astype(np.float32)

        # unique positive pixels (duplicated centers collapse in hm)
        flat = cls * (H * W) + cy * W + cx
        _, uidx = np.unique(flat, return_index=True)
        nu = len(uidx)
        cls_u, cy_u, cx_u = cls[uidx], cy[uidx], cx[uidx]
        inY = (cy_u[None, :] >= y0[:, None]) & (cy_u[None, :] < y1[:, None])
        inX = (cx_u[None, :] >= x0[:, None]) & (cx_u[None, :] < x1[:, None])
        sameC = cls[:, None] == cls_u[None, :]
        Mkj = (sameC & inY & inX).astype(np.float32)  # [k, j<nu]
        npos = Mkj.sum(1)
        MT = np.zeros((K, K), np.float32)
        MT[:nu, :] = Mkj.T
        rpos_v = np.zeros(K, np.int32)
        rpos_v[:nu] = (b % NB) * C * H + cls_u * H + cy_u
        cxsel_v = np.zeros((K, W), np.float32)
        cx_pad = np.zeros(K, np.int64)
        cx_pad[:nu] = cx_u
        cxsel_v[np.arange(K), cx_pad] = 1.0

        r = np.where(npos > 0, 1.0 / np.maximum(npos, 1.0), 1.0)
        s = (-(r * badw * valid)).astype(np.float32)

        rind = ind[b] // W
        cind = ind[b] % W
        EY = (yy[:, None] == rind[None, :]).astype(np.float32)  # [H, K]
        csind_v = np.zeros((K, W), np.float32)
        csind_v[np.arange(K), cind] = 1.0

        m = reg_mask[b]
        blob16[:, _MX : _MX + W] = Mx
        blob16[:, _MT : _MT + K] = MT
        blob16[:, _CXS : _CXS + W] = cxsel_v
        blob16[:, _CSI : _CSI + W] = csind_v
        blob32 = np.zeros((128, _B32), np.float32)
        blob32[:, _EY : _EY + K] = EY
        blob32[:, _GG : _GG + NG] = (
            cls[:, None] // CG == np.arange(NG)[None, :]
        ).astype(np.float32)
        blob32[:, _SK] = s
        blob32[:, _M2] = m
        blob32[:, _M2 + 1] = m
        blob32[:, _TMW : _TMW + 2] = wh_t[b] * m[:, None]
        blob32[:, _TMR : _TMR + 2] = reg_t[b] * m[:, None]
        blob32[:, _RPOS] = rpos_v.view(np.float32)

        nobj = float(m.sum())
        c1 = (1.0 / max(nobj, 1.0)) if nobj > 0 else 1.0
        invden = 1.0 / (2.0 * nobj + 1e-4)

        per_img.append(
            dict(
                blob16=blob16.astype(BF16), blob32=blob32, c1=c1, invden=invden
            )
        )

    in_maps = []
    for core in range(NCORES):
        bs = [core * NB + j for j in range(NB)]
        pi = [per_img[b] for b in bs]
        in_maps.append(
            {
                "phm": np.ascontiguousarray(pred_hm[bs]),
                "hm": np.ascontiguousarray(hm[bs]),
                "pwh": np.ascontiguousarray(pred_wh[bs]),
                "prg": np.ascontiguousarray(pred_reg[bs]),
                "blob16": np.stack([p["blob16"] for p in pi]),
                "blob32": np.stack([p["blob32"] for p in pi]),
            }
        )
    aux = dict(
        c1=np.array([p["c1"] for p in per_img]),
        invden=np.array([p["invden"] for p in per_img]),
    )
    return in_maps, aux


def combine_outputs(outs, aux):
    """outs: list of 8 per-core 'out' arrays [4, NB]."""
    q = np.concatenate([o.T for o in outs], 0).astype(np.float64)  # [B, 4]
    q_hm, q_wh, q_rg = q[:, 0], q[:, 1], q[:, 2]
    wh_i = q_wh * aux["invden"]
    off_i = q_rg * aux["invden"]
    final_loss = np.mean(HM_W * q_hm + WH_W * wh_i + OFF_W * off_i)
    final_hm = np.mean(q_hm * aux["c1"])
    final_wh = np.mean(wh_i)
    final_off = np.mean(off_i)
    return (
        np.float32(final_loss),
        np.float32(final_hm),
        np.float32(final_wh),
        np.float32(final_off),
    )


def kernel(**inputs):
    from concourse.bass_utils import run_bass_kernel_spmd

    nc = build_module()
    in_maps, aux = prep_in_maps(inputs)
    res = run_bass_kernel_spmd(nc, in_maps, core_ids=list(range(NCORES)))
    outs = [r["out"] for r in res.results]
    return combine_outputs(outs, aux)
